# revision 1
# baseline (speedup 1.0000x reference)
"""Bipartite graph attention layer on 8 Trainium2 NeuronCores.

Sharding: data-parallel over (batch b, n_src half). Core c handles
b = c // 2, rows n0 = (c % 2) * 1024 .. +1024 of feat_src; params and
feat_dst[b] replicated per b-pair.

Math (per batch b, head h):
  h_src = feat_src @ W[h]; h_dst = feat_dst @ W[h]
  s[n] = tanh(h_src[n]) . w_src[h];  d[m] = tanh(h_dst[m]) . w_dst[h]
  E[m, n] = exp(leaky_relu(s[n] + d[m], 0.2))
  feat_out[n] = (sum_m E[m,n] h_dst[m]) / (sum_m E[m,n]) + b

Key identity used to avoid materializing logits:
  leaky(x) = 0.2 x + 0.8 relu(x)  =>
  E = exp(0.2 s) * exp(0.2 d) * max(exp(0.8 s) exp(0.8 d), 1)
The per-n factor exp(0.2 s) cancels in the softmax ratio, so the kernel
accumulates E' = E / exp(0.2 s) = max(u8[n] * v1[m], v2[m]) where
u8 = exp(0.8 s), v1 = exp(d), v2 = exp(0.2 d).  Per [128 m, 1024 n]
chunk this is either one DVE tensor_scalar (mult, max), one gpsimd
tensor_scalar, or one ACT relu via E' - v2 = relu(v1 u8 - v2); the
missing v2 term for ACT chunks is restored by one rank-1 correction
matmul per accumulation group with lhsT = the v2 column (its ones-col
entry also fixes the denominator).  sum_m comes from an extra
ones-column appended to the matmul rhs.

All inputs are pre-cast to bf16, transposed, broadcast and packed into
three [128, F] DRAM tensors on the host, so every matmul is bf16, the
kernel issues only 3 input DMAs (DMA issues cost ~0.6-1.2us of SEQ time
each), and there are no on-chip transposes.  a_dst is reduced on the PE
via a block-diagonal wdst rhs against tanh(h_dst^T).  sigmoid(g) is
computed as 0.5 tanh(0.5 g) + 0.5 and elu via relu(y) + exp(min(y,0))-1
so the whole kernel uses one ACT table set.  gpsimd is never used for
compute (real-HW gpsimd ops cost ~7us each regardless of size).
"""

import sys

sys.path.insert(0, "/opt/trn_rl_repo")

import numpy as np

B, N_SRC, N_DST, IN_DIM, OUT_DIM, H = 4, 2048, 2048, 256, 64, 4
N = N_SRC // 2        # n_src rows per core
M = N_DST             # dst rows per core
NT = N // 128         # 8 n-tiles per core
MC = M // 128         # 16 m-chunks
RW = 66               # rhs width: 64 h_dst cols + ones col + pad col
# packed-input column layout (bf16, per 128-partition row):
#   big0 = fdstT [2c x 2048m] ++ W [2c x 256(ho)]
#   big1 = fsrcT [2c x 1024n] ++ wsrc_rep [2pair x 128]
#   big2 = fsrc [8t x 256i] ++ HwT [2c x 256] ++ wdst_blkdiag [4] ++
#          b_bcast [256] ++ Hb_row [256] ++ ones [128]
F0 = 2 * M + 2 * H * OUT_DIM
F1 = 2 * N + 2 * 128
F2 = NT * IN_DIM + 2 * IN_DIM + 4 + 2 * IN_DIM + 128

_CACHE = {}
# Ep chunk engine per (head, mc): D=DVE tensor_scalar, A=ACT relu-form
# (rank-1-corrected), P=gpsimd tensor_scalar.  Head 0 avoids ACT (still
# busy with tanh/exp of the front-end); later heads offload to ACT.
EP_HEAD = [
    "DDDDDDDDDDDDDDDD",
    "DDDDDDDDDDDDDDDD",
    "DADDDADDDDADDDDD",
    "DADDDADDDDADDDDD",
]
RHS_ENGINE = "pool"   # rhs_all psum->sbuf copy engine
ACC_Q = 4             # ns-groups merged per psum accumulator tile


def _build_program(loop=None):
    import concourse.bass as bass
    import concourse.tile as tile
    from concourse import mybir

    f32 = mybir.dt.float32
    bf16 = mybir.dt.bfloat16
    AF = mybir.ActivationFunctionType
    OP = mybir.AluOpType

    nc = bass.Bass()
    big0_d = nc.declare_dram_parameter("big0", [128, F0], bf16, isOutput=False)
    big1_d = nc.declare_dram_parameter("big1", [128, F1], bf16, isOutput=False)
    big2_d = nc.declare_dram_parameter("big2", [128, F2], bf16, isOutput=False)
    out_d = nc.declare_dram_parameter("out", [N, IN_DIM], bf16, isOutput=True)

    with tile.TileContext(nc) as tc:
        if loop is None:
            _emit(nc, tc, bass, mybir, f32, bf16, AF, OP,
                  big0_d, big1_d, big2_d, out_d)
        else:
            with tc.For_i(0, loop):
                _emit(nc, tc, bass, mybir, f32, bf16, AF, OP,
                      big0_d, big1_d, big2_d, out_d)

    _split_sync_waits(nc, mybir)
    return nc


def _emit(nc, tc, bass, mybir, f32, bf16, AF, OP,
          big0_d, big1_d, big2_d, out_d):
    from contextlib import ExitStack

    ctx = ExitStack()
    with ctx:
        const = ctx.enter_context(tc.tile_pool(name="const", bufs=1))
        head_p = ctx.enter_context(tc.tile_pool(name="head", bufs=2))
        ep_p = ctx.enter_context(tc.tile_pool(name="ep", bufs=2))
        fin_p = ctx.enter_context(tc.tile_pool(name="fin", bufs=2))
        ps_tr = ctx.enter_context(tc.tile_pool(name="ps_tr", bufs=2, space="PSUM"))
        ps_hd = ctx.enter_context(tc.tile_pool(name="ps_hd", bufs=2, space="PSUM"))
        ps_acc = ctx.enter_context(tc.tile_pool(name="ps_acc", bufs=3, space="PSUM"))
        ps_q = ctx.enter_context(tc.tile_pool(name="ps_q", bufs=1, space="PSUM"))

        # ------------- loads: 3 packed DMAs -----------
        big0 = const.tile([128, F0], bf16)
        # fdstT arrives as two m-half blocks: the first DMA unblocks the
        # first 8 hd2 chunks ~1.7us before the full transfer lands
        nc.sync.dma_start(big0[:, 0:2048], big0_d[:, 0:2048])
        nc.sync.dma_start(big0[:, 2048:], big0_d[:, 2048:])
        big1 = const.tile([128, F1], bf16)
        nc.scalar.dma_start(big1, big1_d[:, :])
        big2 = const.tile([128, F2], bf16)
        nc.sync.dma_start(big2, big2_d[:, :])

        fdstT_v = big0[:, 0:2 * M].rearrange("p (mh c m) -> p mh c m",
                                             mh=2, c=2)
        W_b = big0[:, 2 * M:].rearrange("p (c h o) -> p c h o", c=2, h=H)
        fsrcT_b = big1[:, 0:2 * N].rearrange("p (c n) -> p c n", c=2)
        wsrc_rep_b = big1[:, 2 * N:].rearrange("p (r w) -> p r w", r=2)
        o2 = NT * IN_DIM
        fsrc_b = big2[:, 0:o2].rearrange("p (t i) -> p t i", t=NT)
        HwT_b = big2[:, o2:o2 + 2 * IN_DIM].rearrange("p (c i) -> p c i", c=2)
        wd_blk = big2[:, o2 + 2 * IN_DIM:o2 + 2 * IN_DIM + 4]
        o3 = o2 + 2 * IN_DIM + 4
        b_bcast_b = big2[:, o3:o3 + IN_DIM].rearrange("p (h o) -> p h o", h=H)
        Hb_row_b = big2[0:1, o3 + IN_DIM:o3 + 2 * IN_DIM]
        onesrow_b = big2[0:1, o3 + 2 * IN_DIM:o3 + 2 * IN_DIM + 128]

        # ------- d-side (h_dst/rhs/tanh) + s-side (u8), interleaved ------
        rhs_all = const.tile([128, H, MC, RW], bf16)
        nc.vector.memset(rhs_all[:, :, :, 64:65], 1.0)
        nc.vector.memset(rhs_all[:, :, :, 65:66], 0.0)
        u8_all = const.tile([128, H, N], bf16)

        def emit_hd2(mp):
            # alternate between two psum pools (same byte size) for an
            # effective depth-4 rotation: PE never waits on the tanh/copy
            # readers more than ~2 tiles back.  gpsimd cannot read PSUM,
            # so the rhs copy alternates DVE/ACT by pair parity.
            pool = ps_hd if mp % 2 == 0 else ps_tr
            hd2 = pool.tile([128, 2, H * OUT_DIM], f32,
                            tag="hd" if mp % 2 == 0 else "tr")
            for k in range(2):
                mc = 2 * mp + k
                for c in range(2):
                    nc.tensor.matmul(
                        hd2[:, k, :],
                        fdstT_v[:, mc // 8, c,
                                128 * (mc % 8):128 * (mc % 8 + 1)],
                        W_b[:, c, :, :].rearrange("p h o -> p (h o)"),
                        start=(c == 0), stop=(c == 1))
            rhs_cp = (nc.vector.tensor_copy if mp % 2 == 0
                      else nc.scalar.copy)
            rhs_cp(rhs_all[:, :, 2 * mp:2 * mp + 2, 0:OUT_DIM],
                   hd2.rearrange("p k (h o) -> p h k o", h=H))

        def emit_hs(pair):
            th_srcT = head_p.tile([128, N], bf16, tag="thsrc")
            for nb in range(2):
                hs = ps_tr.tile([128, 512], f32, tag="tr")
                for c in range(2):
                    nc.tensor.matmul(
                        hs,
                        W_b[:, c, 2 * pair:2 * pair + 2, :].rearrange(
                            "p h o -> p (h o)"),
                        fsrcT_b[:, c, 512 * nb:512 * (nb + 1)],
                        start=(c == 0), stop=(c == 1))
                nc.scalar.activation(th_srcT[:, 512 * nb:512 * (nb + 1)], hs,
                                     AF.Tanh)
            return th_srcT

        def emit_sb(pair, th_srcT):
            for hh in range(2):
                h = 2 * pair + hh
                for nb in range(2):
                    sb = ps_tr.tile([128, 512], f32, tag="tr")
                    nc.tensor.matmul(
                        sb, wsrc_rep_b[64 * hh:64 * (hh + 1), pair, :],
                        th_srcT[64 * hh:64 * (hh + 1),
                                512 * nb:512 * (nb + 1)],
                        start=True, stop=True)
                    nc.scalar.activation(
                        u8_all[:, h, 512 * nb:512 * (nb + 1)], sb, AF.Exp,
                        scale=0.8)

        # a_dst via PE: transposed h_dst (thT) then per-chunk tiny
        # matmuls against a block-diagonal wdst [128, 2] rhs; all 32
        # groups land in one psum tile, one DVE copy extracts them.
        thT = const.tile([128, 2, M], bf16)
        a_dst = const.tile([128, MC, H], bf16)
        v1_all = const.tile([128, MC, H], f32)
        v2_all = const.tile([128, MC, H], f32)
        need_a = any("A" in s for s in EP_HEAD)
        if need_a:
            nv2_all = const.tile([128, MC, H], f32)
            v2c_b = const.tile([128, MC, H], bf16)

        def emit_hdT(half):
            # thT[(hh o), m] = tanh(h_dst^T) for heads (2*half, 2*half+1)
            for mb in range(4):
                ps = ps_tr.tile([128, 512], f32, tag="tr")
                for c in range(2):
                    nc.tensor.matmul(
                        ps,
                        W_b[:, c, 2 * half:2 * half + 2, :].rearrange(
                            "p h o -> p (h o)"),
                        fdstT_v[:, mb // 2, c,
                                512 * (mb % 2):512 * (mb % 2 + 1)],
                        start=(c == 0), stop=(c == 1))
                nc.scalar.activation(thT[:, half, 512 * mb:512 * (mb + 1)],
                                     ps, AF.Tanh)

        def emit_adst():
            aps = ps_hd.tile([128, 2, H * OUT_DIM], f32, tag="hd")
            av = aps.rearrange("p k w -> p (k w)")
            for half in range(2):
                for mc in range(MC):
                    nc.tensor.matmul(
                        av[:, 4 * mc + 2 * half:4 * mc + 2 * half + 2],
                        thT[:, half, 128 * mc:128 * (mc + 1)],
                        wd_blk[:, 2 * half:2 * half + 2],
                        start=True, stop=True)
            return av[:, 0:MC * H].rearrange("p (m h) -> p m h", h=H)

        def emit_exps(av):
            # exps read the a_dst psum directly: one hop less before Ep
            nc.scalar.activation(v1_all, av, AF.Exp)
            nc.scalar.activation(v2_all, av, AF.Exp, scale=0.2)

        def emit_negv2():
            # only needed by the ACT-form Ep chunks of heads 2-3 and
            # their q correction; deferred off the head-0 critical path
            if need_a:
                nc.vector.tensor_scalar(nv2_all, v2_all, -1.0, None, OP.mult)
                nc.vector.tensor_copy(v2c_b, v2_all)

        # d-side tanhs first in the ACT queue (they pace the a_dst
        # reductions); then pair-0 s-side, then the exps.  Pair-1 s-side
        # is emitted after attention head 0 (its PE ops fill idle there).
        emit_hdT(0)
        emit_hdT(1)
        for mp in range(MC // 2):
            emit_hd2(mp)
        av = emit_adst()
        ths0 = emit_hs(0)
        emit_sb(0, ths0)
        emit_exps(av)

        sg_all = const.tile([128, NT, IN_DIM], bf16)

        def emit_gate():
            # gate sigmoid in t-pairs reusing the hd psum tag; tg lands on
            # ACT after the exps, sg on the otherwise-idle Pool engine
            for tp in range(NT // 2):
                gp = ps_hd.tile([128, 2, H * OUT_DIM], f32, tag="hd")
                for k in range(2):
                    t = 2 * tp + k
                    for c in range(2):
                        nc.tensor.matmul(
                            gp[:, k, :], fsrcT_b[:, c, 128 * t:128 * (t + 1)],
                            HwT_b[:, c, :], start=(c == 0), stop=False)
                    nc.tensor.matmul(gp[:, k, :], onesrow_b, Hb_row_b,
                                     start=False, stop=True)
                # sigmoid(g) = 0.5 tanh(0.5 g) + 0.5
                tg = fin_p.tile([128, 2, IN_DIM], bf16, tag="tg")
                nc.scalar.activation(tg, gp, AF.Tanh, scale=0.5)
                nc.vector.tensor_scalar(sg_all[:, 2 * tp:2 * tp + 2, :], tg,
                                        0.5, 0.5, OP.mult, OP.add)

        # -------- heads: Ep chunks + attention, software-pipelined -------
        feat_pre = const.tile([128, NT, H * OUT_DIM], bf16)

        def emit_ep(h, hooks=()):
            hooks = dict(hooks)
            Ep_all = ep_p.tile([128, MC, N], bf16, tag="Ep")
            for mc in range(MC):
                if mc in hooks:
                    hooks[mc]()
                eng = EP_HEAD[h][mc]
                if eng == "A":
                    nc.scalar.activation(Ep_all[:, mc, :], u8_all[:, h, :],
                                         AF.Relu,
                                         bias=nv2_all[:, mc, h:h + 1],
                                         scale=v1_all[:, mc, h:h + 1])
                else:
                    e = nc.vector if eng == "D" else nc.gpsimd
                    e.tensor_scalar(Ep_all[:, mc, :], u8_all[:, h, :],
                                    v1_all[:, mc, h:h + 1],
                                    v2_all[:, mc, h:h + 1], OP.mult, OP.max)
            a_set = [mc for mc in range(MC) if EP_HEAD[h][mc] == "A"]
            q_sb = None
            if a_set:
                q_ps = ps_q.tile([1, RW], f32, tag="q")
                for j, mc in enumerate(a_set):
                    nc.tensor.matmul(q_ps, v2c_b[:, mc, h:h + 1],
                                     rhs_all[:, h, mc, :],
                                     start=(j == 0), stop=(j == len(a_set) - 1))
                q_sb = head_p.tile([1, RW], bf16, tag="qsb")
                nc.vector.tensor_copy(q_sb, q_ps)
            return Ep_all, q_sb

        def emit_attn(h, Ep_all, q_sb):
            accs = []
            for ns4 in range(NT // ACC_Q):
                acc4 = ps_acc.tile([128, ACC_Q * RW], f32, tag="acc")
                for k in range(ACC_Q):
                    ns = ACC_Q * ns4 + k
                    sl = acc4[:, RW * k:RW * (k + 1)]
                    if q_sb is not None:
                        nc.tensor.matmul(sl, onesrow_b, q_sb, start=True,
                                         stop=False)
                    for mc in range(MC):
                        nc.tensor.matmul(
                            sl, Ep_all[:, mc, 128 * ns:128 * (ns + 1)],
                            rhs_all[:, h, mc, :],
                            start=(mc == 0 and q_sb is None),
                            stop=(mc == MC - 1))
                accs.append(acc4)
            return accs

        def emit_epi(h, ns4, acc4):
            accv = acc4.rearrange("p (g w) -> p g w", w=RW)
            rec = fin_p.tile([128, ACC_Q], f32, tag="rec")
            nc.vector.reciprocal(rec, accv[:, :, 64])
            nc.vector.tensor_mul(
                feat_pre[:, ACC_Q * ns4:ACC_Q * (ns4 + 1),
                         OUT_DIM * h:OUT_DIM * (h + 1)],
                accv[:, :, 0:OUT_DIM],
                rec.rearrange("p (g q) -> p g q", q=1).broadcast_to(
                    [128, ACC_Q, OUT_DIM]))

        ep0, q0 = emit_ep(0)
        prev_accs = emit_attn(0, ep0, q0)
        ths1 = emit_hs(1)
        emit_sb(1, ths1)
        for h in range(1, H):
            hooks = {6: (lambda a=prev_accs[0], hp=h - 1: emit_epi(hp, 0, a)),
                     14: (lambda a=prev_accs[1], hp=h - 1: emit_epi(hp, 1, a))}
            if h == 2:
                emit_negv2()
            ep_h, q_h = emit_ep(h, hooks)
            prev_accs = emit_attn(h, ep_h, q_h)
            if h == 1:
                emit_gate()
        xs2 = const.tile([128, NT, IN_DIM], bf16)
        for c2 in range(2):
            t4 = slice(4 * c2, 4 * c2 + 4)
            x1 = fin_p.tile([128, 4, IN_DIM], bf16, tag="x1")
            nc.vector.tensor_scalar(x1, fsrc_b[:, t4, :], 1.0, None, OP.add)
            sx = fin_p.tile([128, 4, IN_DIM], bf16, tag="sx")
            nc.vector.tensor_mul(sx, x1, sg_all[:, t4, :])
            nc.vector.tensor_sub(xs2[:, t4, :], fsrc_b[:, t4, :], sx)
        emit_epi(H - 1, 0, prev_accs[0])
        emit_epi(H - 1, 1, prev_accs[1])

        # ------------- elu + gate + combine (4 pipelined chunks) ---------
        # elu(y) = relu(y) + exp(-relu(-y)) - 1
        # out = x + sg*(elu(y) - x) = [x - sg*(x+1)] + sg*(relu(y) + e1)
        # xs2 = x - sg*(x+1) is precomputed during the head phase; the
        # relu/exp pieces run on ACT so the post-attention DVE chain is
        # just add/mul/add per chunk.
        out_ap = out_d.rearrange("(t p) i -> p t i", p=128)
        bb = b_bcast_b.rearrange("p (q h) o -> p q (h o)", q=1).broadcast_to(
            [128, 2, IN_DIM])
        for c4 in range(4):
            ts = slice(2 * c4, 2 * c4 + 2)
            y = fin_p.tile([128, 2, IN_DIM], bf16, tag="y")
            nc.vector.tensor_add(y, feat_pre[:, ts, :], bb)
            p2 = fin_p.tile([128, 2, IN_DIM], bf16, tag="p2")
            nc.vector.tensor_scalar(p2, y, 0.0, None, OP.max)
            mn = fin_p.tile([128, 2, IN_DIM], bf16, tag="mn")
            nc.vector.tensor_scalar(mn, y, 0.0, None, OP.min)
            e1 = fin_p.tile([128, 2, IN_DIM], bf16, tag="e1")
            nc.scalar.activation(e1, mn, AF.Exp)
            w = fin_p.tile([128, 2, IN_DIM], bf16, tag="w")
            nc.vector.tensor_add(w, p2, e1)
            m3 = fin_p.tile([128, 2, IN_DIM], bf16, tag="m3")
            nc.vector.tensor_mul(m3, w, sg_all[:, ts, :])
            o = fin_p.tile([128, 2, IN_DIM], bf16, tag="o")
            nc.vector.tensor_add(o, m3, xs2[:, ts, :])
            nc.sync.dma_start(out_ap[:, ts, :], o)


def _split_sync_waits(nc, mybir, max_waits=1, drain_max_waits=0):
    """Walrus for cayman here accepts at most one sem-wait per
    instruction (and none on Drain): move overflow waits onto preceding
    same-engine NOPs."""
    n_split = 0
    for f in nc.m.functions:
        for bb in f.blocks:
            il = bb.instructions
            i = 0
            while i < len(il):
                ins = il[i]
                si = ins.sync_info
                limit = (drain_max_waits
                         if type(ins).__name__ == "InstDrain" else max_waits)
                if si is not None and len(si.on_wait) > limit:
                    waits = list(si.on_wait)
                    keep = waits[-limit:] if limit > 0 else []
                    overflow = waits[:len(waits) - limit]
                    chunks = [overflow[j:j + max_waits]
                              for j in range(0, len(overflow), max_waits)]
                    pos = i
                    for chunk in chunks:
                        nop = mybir.InstNoOp(
                            name=f"I-waitsplit-{n_split}",
                            engine=ins.engine,
                            sync_info=mybir.SyncInfo(on_wait=chunk, on_update=[]),
                        )
                        n_split += 1
                        il.insert(pos, nop)
                        pos += 1
                        i += 1
                    ins.sync_info = mybir.SyncInfo(
                        on_wait=keep, on_update=list(si.on_update))
                i += 1
    return n_split


def _get_runner():
    if "runner" in _CACHE:
        return _CACHE["runner"]
    runner = _make_runner(_build_program())
    _CACHE["runner"] = runner
    return runner


def _make_runner(nc):
    import jax
    from jax.sharding import Mesh, PartitionSpec
    from jax.experimental.shard_map import shard_map
    import concourse.mybir as mybir
    from concourse.bass2jax import (_bass_exec_p, install_neuronx_cc_hook,
                                    partition_id_tensor)

    install_neuronx_cc_hook()
    n_cores = 8

    in_names, out_names, out_avals = [], [], []
    for alloc in nc.m.functions[0].allocations:
        if not isinstance(alloc, mybir.MemoryLocationSet):
            continue
        name = alloc.memorylocations[0].name
        if alloc.kind == "ExternalInput":
            if (nc.partition_id_tensor is not None
                    and name == nc.partition_id_tensor.name):
                continue
            in_names.append(name)
        elif alloc.kind == "ExternalOutput":
            out_names.append(name)
            out_avals.append(jax.core.ShapedArray(
                tuple(alloc.tensor_shape), mybir.dt.np(alloc.dtype)))
    n_params = len(in_names)
    in_names_all = list(in_names) + list(out_names)
    if nc.partition_id_tensor is not None:
        in_names_all.append(nc.partition_id_tensor.name)

    def _body(*args):
        operands = list(args)
        if nc.partition_id_tensor is not None:
            operands.append(partition_id_tensor())
        return tuple(_bass_exec_p.bind(
            *operands,
            out_avals=tuple(out_avals),
            in_names=tuple(in_names_all),
            out_names=tuple(out_names),
            lowering_input_output_aliases=(),
            sim_require_finite=True,
            sim_require_nnan=True,
            nc=nc,
        ))

    devices = jax.devices()[:n_cores]
    mesh = Mesh(np.asarray(devices), ("core",))
    n_outs = len(out_names)
    sharded = jax.jit(
        shard_map(_body, mesh=mesh,
                  in_specs=(PartitionSpec("core"),) * (n_params + n_outs),
                  out_specs=(PartitionSpec("core"),) * n_outs,
                  check_rep=False),
        keep_unused=True,
    )
    return (sharded, in_names, out_names, out_avals)


def _shard_inputs(feat_src, feat_dst, W, b, w_src, w_dst, H_w, H_b):
    import ml_dtypes
    bf = ml_dtypes.bfloat16

    def tposed(a2d, width):
        # [R, 256] -> [128, 2, R] -> [128, 2*R]: out[p, c*R + m] = a[m, 128c+p]
        return np.ascontiguousarray(
            a2d.T.reshape(2, 128, width).transpose(1, 0, 2).reshape(128, -1))

    W_pack = W.transpose(1, 0, 2).reshape(2, 128, H * OUT_DIM)
    W_pack = W_pack.transpose(1, 0, 2).reshape(128, -1)
    wsrc_col = w_src.reshape(2, 2, OUT_DIM).transpose(1, 2, 0).reshape(128, 2)
    wsrc_rep = np.repeat(wsrc_col[:, :, None], 128, axis=2).reshape(128, -1)
    HwT = tposed(H_w, IN_DIM)
    wd_blk = np.zeros((128, 4), np.float32)
    for h in range(H):
        wd_blk[64 * (h % 2):64 * (h % 2) + 64, 2 * (h // 2) + h % 2] = w_dst[h]
    b_bcast = np.broadcast_to(np.tile(b, H)[None], (128, H * OUT_DIM))
    Hb_row = np.broadcast_to(H_b[None], (128, IN_DIM))
    ones = np.ones((128, 128), np.float32)

    per_core = []
    for c in range(8):
        bb, half = c // 2, c % 2
        fsrc_c = feat_src[bb, N * half:N * (half + 1)]
        fdst_c = feat_dst[bb]
        big0 = np.concatenate(
            [tposed(fdst_c[0:1024], 1024), tposed(fdst_c[1024:2048], 1024),
             W_pack], axis=1).astype(bf)
        big1 = np.concatenate([tposed(fsrc_c, N), wsrc_rep], axis=1).astype(bf)
        fsrc_norm = fsrc_c.reshape(NT, 128, IN_DIM).transpose(1, 0, 2)
        big2 = np.concatenate(
            [fsrc_norm.reshape(128, -1), HwT, wd_blk, b_bcast, Hb_row,
             ones], axis=1).astype(bf)
        per_core.append({"big0": big0, "big1": big1, "big2": big2})
    return per_core


def kernel(feat_src, feat_dst, W, b, w_src, w_dst, H_w, H_b):
    feat_src = np.asarray(feat_src, np.float32)
    feat_dst = np.asarray(feat_dst, np.float32)
    args = [np.asarray(a, np.float32) for a in (W, b, w_src, w_dst, H_w, H_b)]
    sharded, in_names, out_names, out_avals = _get_runner()
    per_core = _shard_inputs(feat_src, feat_dst, *args)
    concat_in = [np.concatenate([per_core[c][nm] for c in range(8)], axis=0)
                 for nm in in_names]
    concat_zeros = [np.zeros((8 * av.shape[0], *av.shape[1:]), av.dtype)
                    for av in out_avals]
    outs = sharded(*concat_in, *concat_zeros)
    o = np.asarray(outs[out_names.index("out")]).astype(np.float32).reshape(
        8, N, IN_DIM)
    full = np.empty((B, N_SRC, IN_DIM), np.float32)
    for c in range(8):
        bb, half = c // 2, c % 2
        full[bb, N * half:N * (half + 1)] = o[c]
    return full



# revision 11
# speedup vs baseline: 1.7830x; 1.7830x over previous
"""Bipartite graph attention layer on 8 Trainium2 NeuronCores.

Sharding: data-parallel over (batch b, n_src half). Core c handles
b = c // 2, rows n0 = (c % 2) * 1024 .. +1024 of feat_src; params and
feat_dst[b] replicated per b-pair.

Math (per batch b, head h):
  h_src = feat_src @ W[h]; h_dst = feat_dst @ W[h]
  s[n] = tanh(h_src[n]) . w_src[h];  d[m] = tanh(h_dst[m]) . w_dst[h]
  E[m, n] = exp(leaky_relu(s[n] + d[m], 0.2))
  feat_out[n] = (sum_m E[m,n] h_dst[m]) / (sum_m E[m,n]) + b

Key identity used to avoid materializing logits:
  leaky(x) = 0.2 x + 0.8 relu(x)  =>
  E = exp(0.2 s) * exp(0.2 d) * max(exp(0.8 s) exp(0.8 d), 1)
The per-n factor exp(0.2 s) cancels in the softmax ratio, so the kernel
accumulates E' = E / exp(0.2 s) = max(u8[n] * v1[m], v2[m]) where
u8 = exp(0.8 s), v1 = exp(d), v2 = exp(0.2 d).  Per [128 m, 1024 n]
chunk this is either one DVE tensor_scalar (mult, max) or one ACT relu
via E' - v2 = relu(v1 u8 - v2); the missing v2 term for ACT chunks is
restored by one rank-1 correction matmul per accumulation group with
lhsT = the v2 column (its ones-col entry also fixes the denominator).
sum_m comes from an extra ones-column appended to the matmul rhs.

The output bias b is folded into the attention rhs by a rank-1
(ones x b) accumulation into each h_dst projection psum, so the
normalized accumulator directly yields y = feat_out + b.

All inputs are pre-cast to bf16, transposed, broadcast and packed into
four DRAM tensors on the host: bigp (params, loaded first), bigd
(fdstT in four 512-row blocks so early blocks unblock compute), bigs
(fsrcT), bigf (fsrc + HwT).  DMA issues are spread over the sync and
gpsimd queues so transfers overlap; a dependency-free dummy activation
at the top preloads the ACT function table off the critical path.
a_dst is reduced on the PE via a block-diagonal wdst rhs against
tanh(h_dst^T), split per m-half so exp(a_dst) lands early.
sigmoid(g) is computed as 0.5 tanh(0.5 g) + 0.5 and elu via
relu(y) + exp(min(y,0)) - 1 so the whole kernel uses one ACT table
set.  gpsimd is never used for compute (real-HW gpsimd ops cost ~7us
each regardless of size); it only issues software-DGE DMAs.
"""

import sys

sys.path.insert(0, "/opt/trn_rl_repo")

import numpy as np

B, N_SRC, N_DST, IN_DIM, OUT_DIM, H = 4, 2048, 2048, 256, 64, 4
N = N_SRC // 2        # n_src rows per core
M = N_DST             # dst rows per core
NT = N // 128         # 8 n-tiles per core
MC = M // 128         # 16 m-chunks
RW = 66               # rhs width: 64 h_dst cols + ones col + pad col
# packed-input column layout (bf16, per 128-partition row):
#   bigp = W [2c x 256(ho)] ++ wsrc_rep [2pair x 128] ++ wdst_blkdiag [4]
#          ++ b_bcast [256] ++ Hb_row [256] ++ ones [128]
#   bigd = fdstT in four 512-m blocks [4k x 2c x 512m]
#   bigs = fsrcT [2c x 1024n]
#   bigf = fsrc [8t x 256i] ++ HwT [2c x 256]
FP = 2 * H * OUT_DIM + 2 * 128 + 4 + IN_DIM + IN_DIM + 128
FD = 2 * M
FS = 2 * N
FF = NT * IN_DIM + 2 * IN_DIM

_CACHE = {}
# Ep chunk engine per (head, mc): D=DVE tensor_scalar, A=ACT relu-form
# (rank-1-corrected).  Head 0 avoids ACT (still busy with tanh/exp of
# the front-end); later heads offload to ACT.
EP_HEAD = [
    "DDDDDDDDDDDDDDDD",
    "DADDDADDDDADDDDD",
    "DADDDADDDADDDADD",
    "DADDDADDDADDDDDD",
]
ACC_Q = 4             # ns-groups merged per psum accumulator tile


def _build_program(loop=None, split_waits=True):
    import concourse.bass as bass
    import concourse.tile as tile
    from concourse import mybir

    f32 = mybir.dt.float32
    bf16 = mybir.dt.bfloat16
    AF = mybir.ActivationFunctionType
    OP = mybir.AluOpType

    nc = bass.Bass()
    bigp_d = nc.declare_dram_parameter("bigp", [128, FP], bf16, isOutput=False)
    bigd_d = nc.declare_dram_parameter("bigd", [128, FD], bf16, isOutput=False)
    bigs_d = nc.declare_dram_parameter("bigs", [128, FS], bf16, isOutput=False)
    bigf_d = nc.declare_dram_parameter("bigf", [128, FF], bf16, isOutput=False)
    out_d = nc.declare_dram_parameter("out", [N, IN_DIM], bf16, isOutput=True)

    with tile.TileContext(nc) as tc:
        if loop is None:
            _emit(nc, tc, bass, mybir, f32, bf16, AF, OP,
                  bigp_d, bigd_d, bigs_d, bigf_d, out_d)
        elif isinstance(loop, tuple):
            with tc.For_i(0, loop[0]):
                with tc.For_i(0, loop[1]):
                    _emit(nc, tc, bass, mybir, f32, bf16, AF, OP,
                          bigp_d, bigd_d, bigs_d, bigf_d, out_d)
        else:
            with tc.For_i(0, loop):
                _emit(nc, tc, bass, mybir, f32, bf16, AF, OP,
                      bigp_d, bigd_d, bigs_d, bigf_d, out_d)

    if split_waits:
        _split_sync_waits(nc, mybir)
    return nc


def _emit(nc, tc, bass, mybir, f32, bf16, AF, OP,
          bigp_d, bigd_d, bigs_d, bigf_d, out_d):
    from contextlib import ExitStack

    ctx = ExitStack()
    with ctx:
        const = ctx.enter_context(tc.tile_pool(name="const", bufs=1))
        head_p = ctx.enter_context(tc.tile_pool(name="head", bufs=2))
        ep_p = ctx.enter_context(tc.tile_pool(name="ep", bufs=2))
        fin_p = ctx.enter_context(tc.tile_pool(name="fin", bufs=2))
        ps_tr = ctx.enter_context(tc.tile_pool(name="ps_tr", bufs=2, space="PSUM"))
        ps_hd = ctx.enter_context(tc.tile_pool(name="ps_hd", bufs=2, space="PSUM"))
        ps_acc = ctx.enter_context(tc.tile_pool(name="ps_acc", bufs=3, space="PSUM"))
        ps_q = ctx.enter_context(tc.tile_pool(name="ps_q", bufs=1, space="PSUM"))

        # ---- ACT table preload: dep-free dummy activation at t=0 ----
        warm = const.tile([1, 8], bf16)
        nc.vector.memset(warm, 0.0)
        warm2 = const.tile([1, 8], bf16)
        nc.scalar.activation(warm2, warm, AF.Tanh)

        # ---- loads: DMAs ordered by first consumer on the sync queue
        # (gpsimd software-DGE DMAs would parallelize further but their
        # descriptor codegen breaks inside hardware loops) ----
        bigp = const.tile([128, FP], bf16)
        nc.sync.dma_start(bigp, bigp_d[:, :])
        bigd = const.tile([128, FD], bf16)
        nc.sync.dma_start(bigd[:, 0:1024], bigd_d[:, 0:1024])
        bigs = const.tile([128, FS], bf16)
        nc.sync.dma_start(bigs, bigs_d[:, :])
        nc.sync.dma_start(bigd[:, 1024:2048], bigd_d[:, 1024:2048])
        nc.sync.dma_start(bigd[:, 2048:3072], bigd_d[:, 2048:3072])
        nc.sync.dma_start(bigd[:, 3072:4096], bigd_d[:, 3072:4096])
        bigf = const.tile([128, FF], bf16)
        nc.sync.dma_start(bigf, bigf_d[:, :])

        W_b = bigp[:, 0:512].rearrange("p (c h o) -> p c h o", c=2, h=H)
        wsrc_rep_b = bigp[:, 512:768].rearrange("p (r w) -> p r w", r=2)
        wd_blk = bigp[:, 768:772]
        b_row = bigp[0:1, 772:1028]
        Hb_row_b = bigp[0:1, 1028:1284]
        onesrow_b = bigp[0:1, 1284:1412]
        fdstT_v = bigd.rearrange("p (k c m) -> p k c m", k=4, c=2)
        fsrcT_b = bigs.rearrange("p (c n) -> p c n", c=2)
        fsrc_b = bigf[:, 0:NT * IN_DIM].rearrange("p (t i) -> p t i", t=NT)
        HwT_b = bigf[:, NT * IN_DIM:].rearrange("p (c i) -> p c i", c=2)

        # ------- d-side (h_dst/rhs/tanh) + s-side (u8), interleaved ------
        rhs_all = const.tile([128, H, MC, RW], bf16)
        nc.vector.memset(rhs_all[:, :, :, 64:65], 1.0)
        nc.vector.memset(rhs_all[:, :, :, 65:66], 0.0)
        u8_all = const.tile([128, H, N], bf16)

        def emit_hd2(mp):
            # alternate between two psum pools (same byte size) for an
            # effective depth-4 rotation: PE never waits on the tanh/copy
            # readers more than ~2 tiles back.  The rank-1 ones x b
            # matmul folds the output bias into every h_dst row.
            pool = ps_hd if mp % 2 == 0 else ps_tr
            hd2 = pool.tile([128, 2, H * OUT_DIM], f32,
                            tag="hd" if mp % 2 == 0 else "tr")
            for k in range(2):
                mc = 2 * mp + k
                for c in range(2):
                    nc.tensor.matmul(
                        hd2[:, k, :],
                        fdstT_v[:, mc // 4, c,
                                128 * (mc % 4):128 * (mc % 4 + 1)],
                        W_b[:, c, :, :].rearrange("p h o -> p (h o)"),
                        start=(c == 0), stop=False)
                nc.tensor.matmul(hd2[:, k, :], onesrow_b, b_row,
                                 start=False, stop=True)
            rhs_cp = (nc.vector.tensor_copy if mp % 2 == 0
                      else nc.scalar.copy)
            rhs_cp(rhs_all[:, :, 2 * mp:2 * mp + 2, 0:OUT_DIM],
                   hd2.rearrange("p k (h o) -> p h k o", h=H))

        def emit_hs(pair):
            th_srcT = head_p.tile([128, N], bf16, tag="thsrc")
            for nb in range(2):
                hs = ps_tr.tile([128, 512], f32, tag="tr")
                for c in range(2):
                    nc.tensor.matmul(
                        hs,
                        W_b[:, c, 2 * pair:2 * pair + 2, :].rearrange(
                            "p h o -> p (h o)"),
                        fsrcT_b[:, c, 512 * nb:512 * (nb + 1)],
                        start=(c == 0), stop=(c == 1))
                nc.scalar.activation(th_srcT[:, 512 * nb:512 * (nb + 1)], hs,
                                     AF.Tanh)
            return th_srcT

        def emit_sb(pair, th_srcT):
            for hh in range(2):
                h = 2 * pair + hh
                for nb in range(2):
                    sb = ps_tr.tile([128, 512], f32, tag="tr")
                    nc.tensor.matmul(
                        sb, wsrc_rep_b[64 * hh:64 * (hh + 1), pair, :],
                        th_srcT[64 * hh:64 * (hh + 1),
                                512 * nb:512 * (nb + 1)],
                        start=True, stop=True)
                    nc.scalar.activation(
                        u8_all[:, h, 512 * nb:512 * (nb + 1)], sb, AF.Exp,
                        scale=0.8)

        # a_dst via PE: transposed h_dst (thT) then per-chunk tiny
        # matmuls against a block-diagonal wdst [128, 2] rhs, split per
        # m-half so v1/v2 for early m-chunks land before the later
        # tanhs finish.
        thT = const.tile([128, 2, M], bf16)
        a_dst_ps = {}
        v1_all = const.tile([128, MC, H], f32)
        v2_all = const.tile([128, MC, H], f32)
        need_a = any("A" in s for s in EP_HEAD)
        if need_a:
            nv2_all = const.tile([128, MC, H], f32)
            v2c_b = const.tile([128, MC, H], bf16)

        def emit_hdT_mb(half, mb):
            # thT[(hh o), m] block mb = tanh(h_dst^T) for heads
            # (2*half, 2*half+1)
            ps = ps_tr.tile([128, 512], f32, tag="tr")
            for c in range(2):
                nc.tensor.matmul(
                    ps,
                    W_b[:, c, 2 * half:2 * half + 2, :].rearrange(
                        "p h o -> p (h o)"),
                    fdstT_v[:, mb, c, :],
                    start=(c == 0), stop=(c == 1))
            nc.scalar.activation(thT[:, half, 512 * mb:512 * (mb + 1)],
                                 ps, AF.Tanh)

        def emit_adst_exps(half, mh):
            # 8 tiny matmuls -> av [128, 16] psum; exp / exp(0.2 .) into
            # the (mc, head-pair) slices of v1/v2; negated + bf16 copies
            # for the ACT-form Ep chunks.
            aps = ps_q.tile([128, 16], f32, tag="q")
            for j in range(8):
                mc = 8 * mh + j
                nc.tensor.matmul(
                    aps[:, 2 * j:2 * j + 2],
                    thT[:, half, 128 * mc:128 * (mc + 1)],
                    wd_blk[:, 2 * half:2 * half + 2],
                    start=True, stop=True)
            avv = aps.rearrange("p (m h) -> p m h", h=2)
            sl = (slice(8 * mh, 8 * mh + 8), slice(2 * half, 2 * half + 2))
            nc.scalar.activation(v1_all[:, sl[0], sl[1]], avv, AF.Exp)
            nc.scalar.activation(v2_all[:, sl[0], sl[1]], avv, AF.Exp,
                                 scale=0.2)
            if need_a:
                nc.vector.tensor_scalar(nv2_all[:, sl[0], sl[1]],
                                        v2_all[:, sl[0], sl[1]],
                                        -1.0, None, OP.mult)
                nc.vector.tensor_copy(v2c_b[:, sl[0], sl[1]],
                                      v2_all[:, sl[0], sl[1]])

        sg_all = const.tile([128, NT, IN_DIM], bf16)

        def emit_gate():
            # gate sigmoid in t-pairs reusing the hd psum tag; tg lands on
            # ACT after the exps
            for tp in range(NT // 2):
                gp = ps_hd.tile([128, 2, H * OUT_DIM], f32, tag="hd")
                for k in range(2):
                    t = 2 * tp + k
                    for c in range(2):
                        nc.tensor.matmul(
                            gp[:, k, :], fsrcT_b[:, c, 128 * t:128 * (t + 1)],
                            HwT_b[:, c, :], start=(c == 0), stop=False)
                    nc.tensor.matmul(gp[:, k, :], onesrow_b, Hb_row_b,
                                     start=False, stop=True)
                # sigmoid(g) = 0.5 tanh(0.5 g) + 0.5
                tg = fin_p.tile([128, 2, IN_DIM], bf16, tag="tg")
                nc.scalar.activation(tg, gp, AF.Tanh, scale=0.5)
                nc.vector.tensor_scalar(sg_all[:, 2 * tp:2 * tp + 2, :], tg,
                                        0.5, 0.5, OP.mult, OP.add)

        # -------- heads: Ep chunks + attention, software-pipelined -------
        feat_pre = const.tile([128, NT, H * OUT_DIM], bf16)

        def emit_ep(h, hooks=()):
            hooks = dict(hooks)
            Ep_all = ep_p.tile([128, MC, N], bf16, tag="Ep")
            for mc in range(MC):
                if mc in hooks:
                    hooks[mc]()
                eng = EP_HEAD[h][mc]
                if eng == "A":
                    nc.scalar.activation(Ep_all[:, mc, :], u8_all[:, h, :],
                                         AF.Relu,
                                         bias=nv2_all[:, mc, h:h + 1],
                                         scale=v1_all[:, mc, h:h + 1])
                else:
                    nc.vector.tensor_scalar(Ep_all[:, mc, :], u8_all[:, h, :],
                                            v1_all[:, mc, h:h + 1],
                                            v2_all[:, mc, h:h + 1],
                                            OP.mult, OP.max)
            a_set = [mc for mc in range(MC) if EP_HEAD[h][mc] == "A"]
            q_sb = None
            if a_set:
                q_ps = ps_q.tile([1, RW], f32, tag="q")
                for j, mc in enumerate(a_set):
                    nc.tensor.matmul(q_ps, v2c_b[:, mc, h:h + 1],
                                     rhs_all[:, h, mc, :],
                                     start=(j == 0), stop=(j == len(a_set) - 1))
                q_sb = head_p.tile([1, RW], bf16, tag="qsb")
                nc.vector.tensor_copy(q_sb, q_ps)
            return Ep_all, q_sb

        def emit_attn(h, Ep_all, q_sb):
            accs = []
            for ns4 in range(NT // ACC_Q):
                acc4 = ps_acc.tile([128, ACC_Q * RW], f32, tag="acc")
                for k in range(ACC_Q):
                    ns = ACC_Q * ns4 + k
                    sl = acc4[:, RW * k:RW * (k + 1)]
                    if q_sb is not None:
                        nc.tensor.matmul(sl, onesrow_b, q_sb, start=True,
                                         stop=False)
                    for mc in range(MC):
                        nc.tensor.matmul(
                            sl, Ep_all[:, mc, 128 * ns:128 * (ns + 1)],
                            rhs_all[:, h, mc, :],
                            start=(mc == 0 and q_sb is None),
                            stop=(mc == MC - 1))
                accs.append(acc4)
            return accs

        def emit_epi(h, ns4, acc4):
            accv = acc4.rearrange("p (g w) -> p g w", w=RW)
            rec = fin_p.tile([128, ACC_Q], f32, tag="rec")
            nc.vector.reciprocal(rec, accv[:, :, 64])
            nc.vector.tensor_mul(
                feat_pre[:, ACC_Q * ns4:ACC_Q * (ns4 + 1),
                         OUT_DIM * h:OUT_DIM * (h + 1)],
                accv[:, :, 0:OUT_DIM],
                rec.rearrange("p (g q) -> p g q", q=1).broadcast_to(
                    [128, ACC_Q, OUT_DIM]))

        # front-end: head-pair 0 chain first (hdT half 0 -> a_dst/exp per
        # m-half), s-side of pair 0 interleaved so u8 and v1/v2 for head 0
        # finish together; head-pair 1 d-side is deferred past head 0.
        emit_hdT_mb(0, 0)
        emit_hdT_mb(0, 1)
        emit_adst_exps(0, 0)
        ths0 = emit_hs(0)
        emit_sb(0, ths0)
        emit_hdT_mb(0, 2)
        emit_hdT_mb(0, 3)
        emit_adst_exps(0, 1)
        for mp in range(MC // 2):
            emit_hd2(mp)
        # head-pair 1 d-side chain goes on the ACT queue ahead of the
        # head-0/1 attention phase so v1/v2 for heads 2-3 are ready the
        # moment head 2's Ep stream starts (ACT is the pacing engine).
        emit_hdT_mb(1, 0)
        emit_hdT_mb(1, 1)
        emit_adst_exps(1, 0)
        emit_hdT_mb(1, 2)
        emit_hdT_mb(1, 3)
        emit_adst_exps(1, 1)

        ep0, q0 = emit_ep(0)
        prev_accs = emit_attn(0, ep0, q0)
        ths1 = emit_hs(1)
        emit_sb(1, ths1)

        # x + 1 for the gate combine, computed during the head phase
        x1_all = const.tile([128, NT, IN_DIM], bf16)

        def emit_x1():
            for c2 in range(2):
                t4 = slice(4 * c2, 4 * c2 + 4)
                nc.vector.tensor_scalar(x1_all[:, t4, :], fsrc_b[:, t4, :],
                                        1.0, None, OP.add)

        for h in range(1, H):
            hooks = {6: (lambda a=prev_accs[0], hp=h - 1: emit_epi(hp, 0, a)),
                     14: (lambda a=prev_accs[1], hp=h - 1: emit_epi(hp, 1, a))}
            ep_h, q_h = emit_ep(h, hooks)
            prev_accs = emit_attn(h, ep_h, q_h)
            if h == 1:
                emit_gate()
            if h == 2:
                emit_x1()
        emit_epi(H - 1, 0, prev_accs[0])
        emit_epi(H - 1, 1, prev_accs[1])

        # ------------- elu + gate + combine (4 pipelined chunks) ---------
        # y = feat_pre (bias already folded into the attention rhs)
        # elu(y) = relu(y) + exp(min(y,0)) - 1
        # out = x + sg*(elu(y) - x) = x + sg*((relu(y) + e1) - (x + 1))
        out_ap = out_d.rearrange("(t p) i -> p t i", p=128)
        out_q = [nc.sync, nc.sync, nc.sync, nc.scalar]
        for c4 in range(4):
            ts = slice(2 * c4, 2 * c4 + 2)
            y = feat_pre[:, ts, :]
            p2 = fin_p.tile([128, 2, IN_DIM], bf16, tag="p2")
            nc.scalar.activation(p2, y, AF.Relu)
            mn = fin_p.tile([128, 2, IN_DIM], bf16, tag="mn")
            nc.vector.tensor_scalar(mn, y, 0.0, None, OP.min)
            e1 = fin_p.tile([128, 2, IN_DIM], bf16, tag="e1")
            nc.scalar.activation(e1, mn, AF.Exp)
            w = fin_p.tile([128, 2, IN_DIM], bf16, tag="w")
            nc.vector.tensor_add(w, p2, e1)
            wm = fin_p.tile([128, 2, IN_DIM], bf16, tag="wm")
            nc.vector.tensor_sub(wm, w, x1_all[:, ts, :])
            m3 = fin_p.tile([128, 2, IN_DIM], bf16, tag="m3")
            nc.vector.tensor_mul(m3, wm, sg_all[:, ts, :])
            o = fin_p.tile([128, 2, IN_DIM], bf16, tag="o")
            nc.vector.tensor_add(o, m3, fsrc_b[:, ts, :])
            out_q[c4].dma_start(out_ap[:, ts, :], o)


def _split_sync_waits(nc, mybir, max_waits=1, drain_max_waits=0):
    """Walrus for cayman here accepts at most one sem-wait per
    instruction (and none on Drain): move overflow waits onto preceding
    same-engine NOPs."""
    n_split = 0
    for f in nc.m.functions:
        for bb in f.blocks:
            il = bb.instructions
            i = 0
            while i < len(il):
                ins = il[i]
                si = ins.sync_info
                limit = (drain_max_waits
                         if type(ins).__name__ == "InstDrain" else max_waits)
                if si is not None and len(si.on_wait) > limit:
                    waits = list(si.on_wait)
                    keep = waits[-limit:] if limit > 0 else []
                    overflow = waits[:len(waits) - limit]
                    chunks = [overflow[j:j + max_waits]
                              for j in range(0, len(overflow), max_waits)]
                    pos = i
                    for chunk in chunks:
                        nop = mybir.InstNoOp(
                            name=f"I-waitsplit-{n_split}",
                            engine=ins.engine,
                            sync_info=mybir.SyncInfo(on_wait=chunk, on_update=[]),
                        )
                        n_split += 1
                        il.insert(pos, nop)
                        pos += 1
                        i += 1
                    ins.sync_info = mybir.SyncInfo(
                        on_wait=keep, on_update=list(si.on_update))
                i += 1
    return n_split


def _get_runner():
    if "runner" in _CACHE:
        return _CACHE["runner"]
    runner = _make_runner(_build_program())
    _CACHE["runner"] = runner
    return runner


def _make_runner(nc):
    import jax
    from jax.sharding import Mesh, PartitionSpec
    from jax.experimental.shard_map import shard_map
    import concourse.mybir as mybir
    from concourse.bass2jax import (_bass_exec_p, install_neuronx_cc_hook,
                                    partition_id_tensor)

    install_neuronx_cc_hook()
    n_cores = 8

    in_names, out_names, out_avals = [], [], []
    for alloc in nc.m.functions[0].allocations:
        if not isinstance(alloc, mybir.MemoryLocationSet):
            continue
        name = alloc.memorylocations[0].name
        if alloc.kind == "ExternalInput":
            if (nc.partition_id_tensor is not None
                    and name == nc.partition_id_tensor.name):
                continue
            in_names.append(name)
        elif alloc.kind == "ExternalOutput":
            out_names.append(name)
            out_avals.append(jax.core.ShapedArray(
                tuple(alloc.tensor_shape), mybir.dt.np(alloc.dtype)))
    n_params = len(in_names)
    in_names_all = list(in_names) + list(out_names)
    if nc.partition_id_tensor is not None:
        in_names_all.append(nc.partition_id_tensor.name)

    def _body(*args):
        operands = list(args)
        if nc.partition_id_tensor is not None:
            operands.append(partition_id_tensor())
        return tuple(_bass_exec_p.bind(
            *operands,
            out_avals=tuple(out_avals),
            in_names=tuple(in_names_all),
            out_names=tuple(out_names),
            lowering_input_output_aliases=(),
            sim_require_finite=True,
            sim_require_nnan=True,
            nc=nc,
        ))

    devices = jax.devices()[:n_cores]
    mesh = Mesh(np.asarray(devices), ("core",))
    n_outs = len(out_names)
    sharded = jax.jit(
        shard_map(_body, mesh=mesh,
                  in_specs=(PartitionSpec("core"),) * (n_params + n_outs),
                  out_specs=(PartitionSpec("core"),) * n_outs,
                  check_rep=False),
        keep_unused=True,
    )
    return (sharded, in_names, out_names, out_avals)


def _shard_inputs(feat_src, feat_dst, W, b, w_src, w_dst, H_w, H_b):
    import ml_dtypes
    bf = ml_dtypes.bfloat16

    def tposed(a2d, width):
        # [R, 256] -> [128, 2, R] -> [128, 2*R]: out[p, c*R + m] = a[m, 128c+p]
        return np.ascontiguousarray(
            a2d.T.reshape(2, 128, width).transpose(1, 0, 2).reshape(128, -1))

    W_pack = W.transpose(1, 0, 2).reshape(2, 128, H * OUT_DIM)
    W_pack = W_pack.transpose(1, 0, 2).reshape(128, -1)
    wsrc_col = w_src.reshape(2, 2, OUT_DIM).transpose(1, 2, 0).reshape(128, 2)
    wsrc_rep = np.repeat(wsrc_col[:, :, None], 128, axis=2).reshape(128, -1)
    HwT = tposed(H_w, IN_DIM)
    wd_blk = np.zeros((128, 4), np.float32)
    for h in range(H):
        wd_blk[64 * (h % 2):64 * (h % 2) + 64, 2 * (h // 2) + h % 2] = w_dst[h]
    b_bcast = np.broadcast_to(np.tile(b, H)[None], (128, H * OUT_DIM))
    b_pack = b_bcast[:, 0:IN_DIM]
    Hb_row = np.broadcast_to(H_b[None], (128, IN_DIM))
    ones = np.ones((128, 128), np.float32)
    bigp = np.concatenate(
        [W_pack, wsrc_rep, wd_blk, b_pack, Hb_row, ones], axis=1).astype(bf)

    per_core = []
    for c in range(8):
        bb, half = c // 2, c % 2
        fsrc_c = feat_src[bb, N * half:N * (half + 1)]
        fdst_c = feat_dst[bb]
        bigd = np.concatenate(
            [tposed(fdst_c[512 * k:512 * (k + 1)], 512) for k in range(4)],
            axis=1).astype(bf)
        bigs = tposed(fsrc_c, N).astype(bf)
        fsrc_norm = fsrc_c.reshape(NT, 128, IN_DIM).transpose(1, 0, 2)
        bigf = np.concatenate(
            [fsrc_norm.reshape(128, -1), HwT], axis=1).astype(bf)
        per_core.append({"bigp": bigp, "bigd": bigd, "bigs": bigs,
                         "bigf": bigf})
    return per_core


def kernel(feat_src, feat_dst, W, b, w_src, w_dst, H_w, H_b):
    feat_src = np.asarray(feat_src, np.float32)
    feat_dst = np.asarray(feat_dst, np.float32)
    args = [np.asarray(a, np.float32) for a in (W, b, w_src, w_dst, H_w, H_b)]
    sharded, in_names, out_names, out_avals = _get_runner()
    per_core = _shard_inputs(feat_src, feat_dst, *args)
    concat_in = [np.concatenate([per_core[c][nm] for c in range(8)], axis=0)
                 for nm in in_names]
    concat_zeros = [np.zeros((8 * av.shape[0], *av.shape[1:]), av.dtype)
                    for av in out_avals]
    outs = sharded(*concat_in, *concat_zeros)
    o = np.asarray(outs[out_names.index("out")]).astype(np.float32).reshape(
        8, N, IN_DIM)
    full = np.empty((B, N_SRC, IN_DIM), np.float32)
    for c in range(8):
        bb, half = c // 2, c % 2
        full[bb, N * half:N * (half + 1)] = o[c]
    return full
